# revision 36
# baseline (speedup 1.0000x reference)
"""Causal self-attention, tensor-parallel over heads across 8 TRN2 NeuronCores.

B=2, T=2048, C=1024, H=16 heads, D=64. Each core owns 2 heads (128 cols of C)
for both batches, computes QKV projections + causal attention + the softmax
normalizer (via a ones-column appended to v), then two per-head AllToAlls
convert the column-sharded attention output y^T into row shards so each core
computes a disjoint 512-row slice of the final Wo projection; the first A2A
and half the Wo contraction overlap the second head's attention compute.
bf16 matmuls, fp32 PSUM accumulation.
"""

import sys

sys.path.insert(0, "/opt/trn_rl_repo")

import numpy as np
import ml_dtypes

import concourse.bass as bass
import concourse.bacc as bacc
import concourse.mybir as mybir
from concourse.tile import TileContext
from concourse.masks import make_identity
from concourse.tile import add_dep_helper
from concourse import bass_utils

BF16 = mybir.dt.bfloat16
F32 = mybir.dt.float32
NPBF16 = ml_dtypes.bfloat16

B, T, C, H, D = 2, 2048, 1024, 16, 64
NCORES = 8
HL = H // NCORES          # heads per core = 2
COLS = HL * D             # 128 head-cols per core
KT = C // 128             # 8 contraction k-tiles
NCH = T // 512            # 4 query chunks of 512 per batch
NT = T // 128             # 16 key tiles of 128 per batch
VW = D + 1                # 65: v columns + ones column
SH = 68                   # A2A shard rows (64 y + 1 r + pad to 4KB mult)
ROWS_PER_CORE = B * T // NCORES  # 512 output rows per core

MASK_NEG = -60000.0
SCALE = 1.0 / np.sqrt(np.float32(D))
STAGE = "AB2C"
MASK_PE = True
MEMSET_GPS = True

Exp = mybir.ActivationFunctionType.Exp
Copy = mybir.ActivationFunctionType.Copy


def build_nc():
    nc = bacc.Bacc(
        "TRN2",
        target_bir_lowering=False,
        debug=False,
        enable_asserts=False,
        num_devices=NCORES,
    )
    xT = nc.dram_tensor("xT", [C, B * T], BF16, kind="ExternalInput")
    wq = nc.dram_tensor("wq", [C, COLS], BF16, kind="ExternalInput")
    wk = nc.dram_tensor("wk", [C, COLS], BF16, kind="ExternalInput")
    wv = nc.dram_tensor("wv", [C, COLS], BF16, kind="ExternalInput")
    # wo rows pre-permuted on host: h-major [h, core, 64]
    wo = nc.dram_tensor("wo", [C, C], BF16, kind="ExternalInput")
    bqk = nc.dram_tensor("bqk", [COLS, 2], F32, kind="ExternalInput")
    bv = nc.dram_tensor("bv", [1, COLS], BF16, kind="ExternalInput")
    bo = nc.dram_tensor("bo", [1, C], BF16, kind="ExternalInput")
    mtri = nc.dram_tensor("mtri", [128, 128], BF16, kind="ExternalInput")
    emat = nc.dram_tensor("emat", [NCORES, C], BF16, kind="ExternalInput")
    send = [
        nc.dram_tensor(f"a2a_send{h}", [NCORES * SH, ROWS_PER_CORE], BF16)
        for h in range(HL)
    ]
    recv = [
        nc.dram_tensor(f"a2a_recv{h}", [NCORES * SH, ROWS_PER_CORE], BF16)
        for h in range(HL)
    ]
    wsend = nc.dram_tensor("warm_send", [NCORES * 2, 512], BF16)
    wrecv = nc.dram_tensor("warm_recv", [NCORES * 2, 512], BF16)
    out = nc.dram_tensor("out", [ROWS_PER_CORE, C], F32, kind="ExternalOutput")

    add = mybir.AluOpType.add
    mult = mybir.AluOpType.mult

    def load_w3(dst, src, width):
        """One DMA: dst [128, KT*width] <- src [C, width] with k-tile blocks."""
        nc.sync.dma_start(
            out=dst.rearrange("p (k w) -> p k w", k=KT),
            in_=src.rearrange("(k p) w -> p k w", k=KT),
        )

    def stage_a(x_sb, psA, cs):
        for b in range(B):
            for proj, wsb, dsb, bcol in (
                ("q", cs["wq"], cs["qTd"], 0),
                ("k", cs["wk"], cs["kTd"], 1),
            ):
                for n in range(NCH):
                    ps = psA.tile([128, 512], F32, tag="psA", name="psA")
                    col = b * T + n * 512
                    for k in range(KT):
                        nc.tensor.matmul(
                            ps[:],
                            wsb[:, k * COLS : (k + 1) * COLS],
                            x_sb[k][:, col : col + 512],
                            start=(k == 0),
                            stop=(k == KT - 1),
                        )
                    # write each head into BOTH halves of its dup tile so
                    # score matmul pairs can use disjoint PE row-groups
                    for h in range(HL):
                        for half in range(2):
                            nc.vector.tensor_scalar(
                                dsb[h][half * D : half * D + D, col : col + 512],
                                ps[h * D : (h + 1) * D, :],
                                cs["bqk"][h * D : (h + 1) * D, bcol : bcol + 1],
                                None,
                                add,
                            )
            # v: natural layout [T-tile, cols]
            for m in range(NT):
                ps = psA.tile([128, COLS], F32, tag="psAv", name="psAv")
                col = b * T + m * 128
                for k in range(KT):
                    nc.tensor.matmul(
                        ps[:],
                        x_sb[k][:, col : col + 128],
                        cs["wv"][:, k * COLS : (k + 1) * COLS],
                        start=(k == 0),
                        stop=False,
                    )
                nc.tensor.matmul(
                    ps[:],
                    cs["ones"][0:1, :],
                    cs["bv"][0:1, :],
                    start=False,
                    stop=True,
                )
                # v_sb [128, B*NT*2, VW]; ones column (idx D) preset by memset
                vi = (b * NT + m) * HL
                nc.vector.tensor_copy(
                    out=cs["v"][:, vi : vi + HL, 0:D],
                    in_=ps[:, :].rearrange("p (h d) -> p h d", h=HL),
                )

    def stage_b_head(h, ptp, psS, psY, nrm, cs, send_instrs):
        for b in range(B):
            for n in range(NCH):
                qcol = b * T + n * 512
                qTd, kTd = cs["qTd"][h], cs["kTd"][h]
                pts = []
                for m0 in range(0, 4 * n, 2):
                    # two key tiles share one PSUM pair-tile and one exp; the
                    # two matmuls use disjoint PE row-groups and overlap
                    ps2 = psS.tile([128, 1024], F32, tag="psS2", name="psS2", bufs=2)
                    pt2 = ptp.tile([128, 1024], BF16, tag="pt2", name="pt2", bufs=20)
                    for i in range(2):
                        kcol = b * T + (m0 + i) * 128
                        nc.tensor.matmul(
                            ps2[:, i * 512 : (i + 1) * 512],
                            kTd[i * D : (i + 1) * D, kcol : kcol + 128],
                            qTd[i * D : (i + 1) * D, qcol : qcol + 512],
                            start=True,
                            stop=True,
                        )
                    nc.scalar.activation(pt2[:], ps2[:], Exp, scale=float(SCALE))
                    pts.append(pt2[:, 0:512])
                    pts.append(pt2[:, 512:1024])
                for m in range(4 * n, 4 * n + 4):
                    pt = ptp.tile([128, 512], BF16, tag="pt", name="pt")
                    ps = psS.tile([128, 512], F32, tag="psS", name="psS", bufs=1)
                    kcol = b * T + m * 128
                    if True:
                        j = m - 4 * n
                        # NOTE: all diag sub-MMs share one PSUM bank, so they
                        # must use ONE row-group (concurrent row-group writes
                        # to the same bank fault the PE)
                        for idx, sq in enumerate(range(j, 4)):
                            half = 0
                            nc.tensor.matmul(
                                ps[:, sq * 128 : (sq + 1) * 128],
                                kTd[half * D : (half + 1) * D, kcol : kcol + 128],
                                qTd[
                                    half * D : (half + 1) * D,
                                    qcol + sq * 128 : qcol + (sq + 1) * 128,
                                ],
                                start=True,
                                stop=(sq != j or not MASK_PE),
                            )
                            if sq == j and MASK_PE:
                                # causal mask of the diagonal block via PE:
                                # ps[:, j] += I.T @ mtri (adjacent in the
                                # accumulation group so no other start=True
                                # clears the region's has_written bits)
                                nc.tensor.matmul(
                                    ps[:, j * 128 : (j + 1) * 128],
                                    cs["ident"][:],
                                    cs["mtri"][:],
                                    start=False,
                                    stop=True,
                                )
                        if not MASK_PE:
                            nc.vector.tensor_tensor(
                                ps[:, j * 128 : (j + 1) * 128],
                                ps[:, j * 128 : (j + 1) * 128],
                                cs["mtri"][:],
                                add,
                            )
                        if j > 0:
                            (nc.gpsimd if MEMSET_GPS else nc.vector).memset(
                                pt[:, 0 : j * 128], 0.0
                            )
                        nc.scalar.activation(
                            pt[:, j * 128 :],
                            ps[:, j * 128 :],
                            Exp,
                            scale=float(SCALE),
                        )
                    pts.append(pt)

                # y^T (+ sum row) = v_aug^T @ P^T
                py = psY.tile([VW, 512], F32, tag="psY", name="psY")
                last = 4 * n + 3
                for m in range(4 * n + 4):
                    vi = (b * NT + m) * HL + h
                    nc.tensor.matmul(
                        py[:],
                        cs["v"][:, vi : vi + 1, :],
                        pts[m][:],
                        start=(m == 0),
                        stop=(m == last),
                    )
                # ship unnormalized y^T plus the r row; receive side divides
                yn = nrm.tile([VW, 512], BF16, tag="yn", name="yn")
                nc.vector.tensor_copy(out=yn[:], in_=py[:])
                shard = 4 * b + n
                si = nc.sync.dma_start(
                    out=send[h][shard * SH : shard * SH + VW, :], in_=yn[:]
                )
                send_instrs.append(si)

    def a2a(h):
        nc.gpsimd.collective_compute(
            "AllToAll",
            mybir.AluOpType.bypass,
            replica_groups=[list(range(NCORES))],
            ins=[send[h][:]],
            outs=[recv[h][:]],
        )

    def stage_c_half(h, cp, psC, cs, acc, after):
        """Wo contraction over the 4 k-tiles supplied by recv[h]."""
        y_sb = cp.tile([128, 4 * 512], BF16, tag=f"ysb{h}", name=f"ysb{h}")
        r_sb = cp.tile([NCORES, 512], BF16, tag=f"rsb{h}", name=f"rsb{h}")
        dmas = []
        for s in range(NCORES):
            dmas.append(
                nc.sync.dma_start(
                    out=y_sb[(s % 2) * D : (s % 2) * D + D,
                             (s // 2) * 512 : (s // 2) * 512 + 512],
                    in_=recv[h][s * SH : s * SH + D, :],
                )
            )
        for s in range(NCORES):
            dmas.append(
                nc.sync.dma_start(
                    out=r_sb[s : s + 1, :],
                    in_=recv[h][s * SH + D : s * SH + D + 1, :],
                )
            )
        for dd in dmas:
            # keep unpack DMAs behind the attention sends on their queues:
            # the scheduler otherwise queues them early and the
            # collective-completion wait blocks later sends
            add_dep_helper(
                dd.ins, after.ins, reason="unpack after attention sends"
            )
        inv = cp.tile([NCORES, 512], BF16, tag=f"inv{h}", name=f"inv{h}")
        with nc.allow_low_precision(reason="bf16 softmax norm, tol 2e-2"):
            nc.vector.reciprocal(inv[:], r_sb[:])
        yn_sb = cp.tile([128, 4 * 512], BF16, tag=f"ynsb{h}", name=f"ynsb{h}")
        for k in range(4):
            pb = psC.tile([128, 512], F32, tag="psC", name="psBC")
            nc.tensor.matmul(
                pb[:],
                cs["emat"][:, k * 128 : (k + 1) * 128],
                inv[:],
                start=True,
                stop=True,
            )
            nc.vector.tensor_tensor(
                yn_sb[:, k * 512 : (k + 1) * 512],
                y_sb[:, k * 512 : (k + 1) * 512],
                pb[:],
                mult,
            )
        y_sb = yn_sb
        for r in range(ROWS_PER_CORE // 128):
            for o in range(C // 512):
                pc = psC.tile([128, 512], F32, tag="psC", name="psC")
                for k in range(4):
                    kk = h * 4 + k
                    nc.tensor.matmul(
                        pc[:],
                        y_sb[:, k * 512 + r * 128 : k * 512 + r * 128 + 128],
                        cs["wo"][:, kk * C + o * 512 : kk * C + (o + 1) * 512],
                        start=(k == 0),
                        stop=(h == 1 and k == 3),
                    )
                if h == 0:
                    nc.tensor.matmul(
                        pc[:],
                        cs["ones"][0:1, :],
                        cs["bo"][0:1, o * 512 : (o + 1) * 512],
                        start=False,
                        stop=True,
                    )
                    a = cp.tile([128, 512], F32, tag="acc", name="acc", bufs=8)
                    nc.vector.tensor_copy(out=a[:], in_=pc[:])
                    acc[(r, o)] = a
                else:
                    osb = cp.tile([128, 512], F32, tag="osb", name="osb", bufs=3)
                    nc.vector.tensor_tensor(osb[:], pc[:], acc[(r, o)][:], add)
                    nc.sync.dma_start(
                        out=out[r * 128 : (r + 1) * 128, o * 512 : (o + 1) * 512],
                        in_=osb[:],
                    )

    with TileContext(nc) as tc:
        with tc.tile_pool(name="persist", bufs=1) as pp:
            cs = {}
            # weights first (needed with first x tiles)
            cs["wq"] = pp.tile([128, KT * COLS], BF16, tag="wq", name="wq")
            cs["wk"] = pp.tile([128, KT * COLS], BF16, tag="wk", name="wk")
            cs["wv"] = pp.tile([128, KT * COLS], BF16, tag="wv", name="wv")
            load_w3(cs["wq"], wq[:], COLS)
            load_w3(cs["wk"], wk[:], COLS)
            load_w3(cs["wv"], wv[:], COLS)

            with tc.tile_pool(name="xp", bufs=1) as xp:
                x_sb = []
                xq = [nc.sync, nc.scalar, nc.gpsimd]
                for k in range(KT):
                    xt = xp.tile([128, B * T], BF16, tag=f"x{k}", name=f"x{k}")
                    xq[k % 3].dma_start(out=xt[:], in_=xT[k * 128 : (k + 1) * 128, :])
                    x_sb.append(xt)

                # small constants (issued on scalar queue to not delay x)
                cs["ones"] = pp.tile([1, 128], BF16, tag="ones", name="ones")
                nc.vector.memset(cs["ones"][:], 1.0)
                cs["bqk"] = pp.tile([COLS, 2], F32, tag="bqk", name="bqk")
                nc.scalar.dma_start(out=cs["bqk"][:], in_=bqk[:])
                cs["bv"] = pp.tile([1, COLS], BF16, tag="bv", name="bv")
                nc.scalar.dma_start(out=cs["bv"][:], in_=bv[:])
                cs["bo"] = pp.tile([1, C], BF16, tag="bo", name="bo")
                nc.scalar.dma_start(out=cs["bo"][:], in_=bo[:])
                cs["mtri"] = pp.tile([128, 128], BF16, tag="mtri", name="mtri")
                nc.scalar.dma_start(out=cs["mtri"][:], in_=mtri[:])
                cs["emat"] = pp.tile([NCORES, C], BF16, tag="emat", name="emat")
                nc.scalar.dma_start(out=cs["emat"][:], in_=emat[:])
                cs["ident"] = pp.tile([128, 128], BF16, tag="ident", name="ident")
                make_identity(nc, cs["ident"][:])

                cs["qTd"] = [
                    pp.tile([128, B * T], BF16, tag=f"qTd{h}", name=f"qTd{h}")
                    for h in range(HL)
                ]
                cs["kTd"] = [
                    pp.tile([128, B * T], BF16, tag=f"kTd{h}", name=f"kTd{h}")
                    for h in range(HL)
                ]
                cs["v"] = pp.tile([128, B * NT * HL, VW], BF16, tag="v", name="v")
                nc.gpsimd.memset(cs["v"][:], 1.0)  # presets the ones columns

                # wo loaded last (not needed until stage C)
                cs["wo"] = pp.tile([128, KT * C], BF16, tag="wo", name="wo")
                nc.scalar.dma_start(
                    out=cs["wo"].rearrange("p (k w) -> p k w", k=KT),
                    in_=wo.rearrange("(k p) w -> p k w", k=KT),
                )

                with tc.tile_pool(name="psA", bufs=4, space="PSUM") as psA:
                    if "A" in STAGE:
                        stage_a(x_sb, psA, cs)

            acc = {}
            if "B" in STAGE:
                with tc.tile_pool(name="pt", bufs=40) as ptp, tc.tile_pool(
                    name="psS", bufs=1, space="PSUM"
                ) as psS, tc.tile_pool(
                    name="psY", bufs=1, space="PSUM"
                ) as psY, tc.tile_pool(
                    name="nrm", bufs=4
                ) as nrm, tc.tile_pool(
                    name="cp", bufs=1
                ) as cp, tc.tile_pool(
                    name="psC", bufs=2, space="PSUM"
                ) as psC:
                    send_instrs = []
                    if "2" in STAGE:
                        # warmup: absorb the ~11us first-collective trigger
                        # latency while projections run
                        nc.gpsimd.collective_compute(
                            "AllToAll",
                            mybir.AluOpType.bypass,
                            replica_groups=[list(range(NCORES))],
                            ins=[wsend[:]],
                            outs=[wrecv[:]],
                        )
                    stage_b_head(0, ptp, psS, psY, nrm, cs, send_instrs)
                    if "2" in STAGE:
                        a2a(0)
                    stage_b_head(1, ptp, psS, psY, nrm, cs, send_instrs)
                    if "2" in STAGE:
                        a2a(1)
                        if "C" in STAGE:
                            stage_c_half(0, cp, psC, cs, acc, send_instrs[11])
                            stage_c_half(1, cp, psC, cs, acc, send_instrs[-1])
    nc.compile()
    return nc


def make_in_maps(x, mask, Wq, bq, Wk, bk, Wv, bv, Wo, bo):
    xT = np.ascontiguousarray(
        x.astype(np.float32).transpose(2, 0, 1).reshape(C, B * T)
    ).astype(NPBF16)
    mtri = np.where(
        np.arange(128)[:, None] > np.arange(128)[None, :], MASK_NEG, 0.0
    ).astype(NPBF16)
    # Wo rows permuted h-major: new row order = [core0 h0 d0..63, core1 h0, ...,
    # core7 h0, core0 h1, ..., core7 h1]
    perm = np.concatenate(
        [
            np.arange(c * COLS + h * D, c * COLS + h * D + D)
            for h in range(HL)
            for c in range(NCORES)
        ]
    )
    wo_b = np.ascontiguousarray(Wo[perm]).astype(NPBF16)
    # emat[s, c] = 1 iff shard s supplies the softmax normalizer for y-column
    # position c of the per-head k-chunk layout (chunk k = shards 2k, 2k+1)
    cols = np.arange(C)
    emat_np = (
        np.arange(NCORES)[:, None] == (2 * (cols // 128) + (cols % 128) // 64)
    ).astype(NPBF16)
    bo_b = bo.reshape(1, C).astype(NPBF16)
    in_maps = []
    for c in range(NCORES):
        cslice = slice(c * COLS, (c + 1) * COLS)
        in_maps.append(
            {
                "xT": xT,
                "wq": np.ascontiguousarray(Wq[:, cslice]).astype(NPBF16),
                "wk": np.ascontiguousarray(Wk[:, cslice]).astype(NPBF16),
                "wv": np.ascontiguousarray(Wv[:, cslice]).astype(NPBF16),
                "wo": wo_b,
                "bqk": np.stack([bq[cslice], bk[cslice]], axis=1).astype(
                    np.float32
                ),
                "bv": bv[cslice].reshape(1, COLS).astype(NPBF16),
                "bo": bo_b,
                "mtri": mtri,
                "emat": emat_np,
            }
        )
    return in_maps


_CACHED_NC = None


def run(inputs, trace=False, **kw):
    global _CACHED_NC
    if _CACHED_NC is None:
        _CACHED_NC = build_nc()
    in_maps = make_in_maps(**inputs)
    res = bass_utils.run_bass_kernel_spmd(
        _CACHED_NC, in_maps, core_ids=list(range(NCORES)), trace=trace, **kw
    )
    outs = [np.asarray(res.results[c]["out"]) for c in range(NCORES)]
    full = np.concatenate(outs, axis=0).reshape(B, T, C).astype(np.float32)
    return full, res


def kernel(**inputs):
    full, _ = run(inputs, trace=False)
    return full


# revision 42
# speedup vs baseline: 1.0767x; 1.0767x over previous
"""Causal self-attention, tensor-parallel over heads across 8 TRN2 NeuronCores.

B=2, T=2048, C=1024, H=16 heads, D=64. Each core owns 2 heads (128 cols of C)
for both batches, computes QKV projections + causal attention + the softmax
normalizer (via a ones-column appended to v), then two per-head AllToAlls
convert the column-sharded attention output y^T into row shards so each core
computes a disjoint 512-row slice of the final Wo projection; the first A2A
and half the Wo contraction overlap the second head's attention compute.
bf16 matmuls, fp32 PSUM accumulation.
"""

import sys

sys.path.insert(0, "/opt/trn_rl_repo")

import numpy as np
import ml_dtypes

import concourse.bass as bass
import concourse.bacc as bacc
import concourse.mybir as mybir
from concourse.tile import TileContext
from concourse.masks import make_identity
from concourse.tile import add_dep_helper
from concourse import bass_utils

BF16 = mybir.dt.bfloat16
F32 = mybir.dt.float32
NPBF16 = ml_dtypes.bfloat16

B, T, C, H, D = 2, 2048, 1024, 16, 64
NCORES = 8
HL = H // NCORES          # heads per core = 2
COLS = HL * D             # 128 head-cols per core
KT = C // 128             # 8 contraction k-tiles
NCH = T // 512            # 4 query chunks of 512 per batch
NT = T // 128             # 16 key tiles of 128 per batch
VW = D + 1                # 65: v columns + ones column
SH = 72                   # A2A shard rows (64 y + 1 r + pad to whole 4KB)
ROWS_PER_CORE = B * T // NCORES  # 512 output rows per core

MASK_NEG = -60000.0
SCALE = 1.0 / np.sqrt(np.float32(D))
STAGE = "AB2C"
MASK_PE = True
MEMSET_GPS = True

Exp = mybir.ActivationFunctionType.Exp
Copy = mybir.ActivationFunctionType.Copy


def build_nc():
    nc = bacc.Bacc(
        "TRN2",
        target_bir_lowering=False,
        debug=False,
        enable_asserts=False,
        num_devices=NCORES,
    )
    xT = nc.dram_tensor("xT", [C, B * T], BF16, kind="ExternalInput")
    wq = nc.dram_tensor("wq", [C, COLS], BF16, kind="ExternalInput")
    wk = nc.dram_tensor("wk", [C, COLS], BF16, kind="ExternalInput")
    wv = nc.dram_tensor("wv", [C, COLS], BF16, kind="ExternalInput")
    # wo rows pre-permuted on host: h-major [h, core, 64]
    wo = nc.dram_tensor("wo", [C, C], BF16, kind="ExternalInput")
    bqk = nc.dram_tensor("bqk", [COLS, 2], F32, kind="ExternalInput")
    bv = nc.dram_tensor("bv", [1, COLS], BF16, kind="ExternalInput")
    bo = nc.dram_tensor("bo", [1, C], BF16, kind="ExternalInput")
    mtri = nc.dram_tensor("mtri", [128, 128], BF16, kind="ExternalInput")
    emat = nc.dram_tensor("emat", [NCORES, C], BF16, kind="ExternalInput")
    # 75/25 exchange: A2A_0 carries h0 (both batches) + h1 batch0; A2A_1 only
    # h1 batch1. Core j owns rows [256j:256j+256] of EACH batch, so every
    # chunk supplies all 8 destinations and shards stay uniform.
    send = [
        nc.dram_tensor("a2a_send0", [NCORES * SH, 768], BF16),
        nc.dram_tensor("a2a_send1", [NCORES * SH, 256], BF16),
    ]
    recv = [
        nc.dram_tensor("a2a_recv0", [NCORES * SH, 768], BF16),
        nc.dram_tensor("a2a_recv1", [NCORES * SH, 256], BF16),
    ]
    wsend = nc.dram_tensor("warm_send", [NCORES * 2, 512], BF16)
    wrecv = nc.dram_tensor("warm_recv", [NCORES * 2, 512], BF16)
    out = nc.dram_tensor("out", [ROWS_PER_CORE, C], F32, kind="ExternalOutput")

    add = mybir.AluOpType.add
    mult = mybir.AluOpType.mult

    def load_w3(dst, src, width):
        """One DMA: dst [128, KT*width] <- src [C, width] with k-tile blocks."""
        nc.sync.dma_start(
            out=dst.rearrange("p (k w) -> p k w", k=KT),
            in_=src.rearrange("(k p) w -> p k w", k=KT),
        )

    def stage_a(x_sb, psA, cs):
        for b in range(B):
            for proj, wsb, dsb, bcol in (
                ("q", cs["wq"], cs["qTd"], 0),
                ("k", cs["wk"], cs["kTd"], 1),
            ):
                for n in range(NCH):
                    ps = psA.tile([128, 512], F32, tag="psA", name="psA")
                    col = b * T + n * 512
                    for k in range(KT):
                        nc.tensor.matmul(
                            ps[:],
                            wsb[:, k * COLS : (k + 1) * COLS],
                            x_sb[k][:, col : col + 512],
                            start=(k == 0),
                            stop=(k == KT - 1),
                        )
                    # write each head into BOTH halves of its dup tile so
                    # score matmul pairs can use disjoint PE row-groups
                    for h in range(HL):
                        for half in range(2):
                            nc.vector.tensor_scalar(
                                dsb[h][half * D : half * D + D, col : col + 512],
                                ps[h * D : (h + 1) * D, :],
                                cs["bqk"][h * D : (h + 1) * D, bcol : bcol + 1],
                                None,
                                add,
                            )
            # v: natural layout [T-tile, cols]
            for m in range(NT):
                ps = psA.tile([128, COLS], F32, tag="psAv", name="psAv")
                col = b * T + m * 128
                for k in range(KT):
                    nc.tensor.matmul(
                        ps[:],
                        x_sb[k][:, col : col + 128],
                        cs["wv"][:, k * COLS : (k + 1) * COLS],
                        start=(k == 0),
                        stop=False,
                    )
                nc.tensor.matmul(
                    ps[:],
                    cs["ones"][0:1, :],
                    cs["bv"][0:1, :],
                    start=False,
                    stop=True,
                )
                # v_sb [128, B*NT*2, VW]; ones column (idx D) preset by memset
                vi = (b * NT + m) * HL
                nc.vector.tensor_copy(
                    out=cs["v"][:, vi : vi + HL, 0:D],
                    in_=ps[:, :].rearrange("p (h d) -> p h d", h=HL),
                )

    def stage_b_head(h, b_list, ptp, psS, psY, nrm, cs, send_instrs):
        for b in b_list:
            for n in range(NCH):
                qcol = b * T + n * 512
                qTd, kTd = cs["qTd"][h], cs["kTd"][h]
                pts = []
                for m0 in range(0, 4 * n, 2):
                    # two key tiles share one PSUM pair-tile and one exp; the
                    # two matmuls use disjoint PE row-groups and overlap
                    ps2 = psS.tile([128, 1024], F32, tag="psS2", name="psS2", bufs=2)
                    pt2 = ptp.tile([128, 1024], BF16, tag="pt2", name="pt2", bufs=20)
                    for i in range(2):
                        kcol = b * T + (m0 + i) * 128
                        nc.tensor.matmul(
                            ps2[:, i * 512 : (i + 1) * 512],
                            kTd[i * D : (i + 1) * D, kcol : kcol + 128],
                            qTd[i * D : (i + 1) * D, qcol : qcol + 512],
                            start=True,
                            stop=True,
                        )
                    nc.scalar.activation(pt2[:], ps2[:], Exp, scale=float(SCALE))
                    pts.append(pt2[:, 0:512])
                    pts.append(pt2[:, 512:1024])
                for m in range(4 * n, 4 * n + 4):
                    pt = ptp.tile([128, 512], BF16, tag="pt", name="pt")
                    ps = psS.tile([128, 512], F32, tag="psS", name="psS", bufs=1)
                    kcol = b * T + m * 128
                    if True:
                        j = m - 4 * n
                        # NOTE: all diag sub-MMs share one PSUM bank, so they
                        # must use ONE row-group (concurrent row-group writes
                        # to the same bank fault the PE)
                        for idx, sq in enumerate(range(j, 4)):
                            half = 0
                            nc.tensor.matmul(
                                ps[:, sq * 128 : (sq + 1) * 128],
                                kTd[half * D : (half + 1) * D, kcol : kcol + 128],
                                qTd[
                                    half * D : (half + 1) * D,
                                    qcol + sq * 128 : qcol + (sq + 1) * 128,
                                ],
                                start=True,
                                stop=(sq != j or not MASK_PE),
                            )
                            if sq == j and MASK_PE:
                                # causal mask of the diagonal block via PE:
                                # ps[:, j] += I.T @ mtri (adjacent in the
                                # accumulation group so no other start=True
                                # clears the region's has_written bits)
                                nc.tensor.matmul(
                                    ps[:, j * 128 : (j + 1) * 128],
                                    cs["ident"][:],
                                    cs["mtri"][:],
                                    start=False,
                                    stop=True,
                                )
                        if not MASK_PE:
                            nc.vector.tensor_tensor(
                                ps[:, j * 128 : (j + 1) * 128],
                                ps[:, j * 128 : (j + 1) * 128],
                                cs["mtri"][:],
                                add,
                            )
                        if j > 0:
                            (nc.gpsimd if MEMSET_GPS else nc.vector).memset(
                                pt[:, 0 : j * 128], 0.0
                            )
                        nc.scalar.activation(
                            pt[:, j * 128 :],
                            ps[:, j * 128 :],
                            Exp,
                            scale=float(SCALE),
                        )
                    pts.append(pt)

                # y^T (+ sum row) = v_aug^T @ P^T
                py = psY.tile([VW, 512], F32, tag="psY", name="psY")
                last = 4 * n + 3
                for m in range(4 * n + 4):
                    vi = (b * NT + m) * HL + h
                    nc.tensor.matmul(
                        py[:],
                        cs["v"][:, vi : vi + 1, :],
                        pts[m][:],
                        start=(m == 0),
                        stop=(m == last),
                    )
                # ship unnormalized y^T plus the r row; receive side divides.
                # chunk (b, n) rows [256p] belong to destination core 2n+p.
                yn = nrm.tile([VW, 512], BF16, tag="yn", name="yn")
                nc.vector.tensor_copy(out=yn[:], in_=py[:])
                for p in range(2):
                    dst = 2 * n + p
                    if h == 0:
                        o = send[0][dst * SH : dst * SH + VW, b * 256 : b * 256 + 256]
                    elif b == 0:
                        o = send[0][dst * SH : dst * SH + VW, 512:768]
                    else:
                        o = send[1][dst * SH : dst * SH + VW, 0:256]
                    si = nc.sync.dma_start(
                        out=o, in_=yn[:, p * 256 : (p + 1) * 256]
                    )
                    send_instrs.append(si)

    def a2a(h):
        nc.gpsimd.collective_compute(
            "AllToAll",
            mybir.AluOpType.bypass,
            replica_groups=[list(range(NCORES))],
            ins=[send[h][:]],
            outs=[recv[h][:]],
        )

    def unpack_dep(dmas, after):
        for dd in dmas:
            # keep unpack DMAs behind the attention sends on the Sync queue:
            # the scheduler otherwise queues them early and their
            # collective-completion wait blocks later sends
            add_dep_helper(
                dd.ins, after.ins, reason="unpack after attention sends"
            )

    def make_inv(cp, r_sb, tag):
        r_f = cp.tile([NCORES, 512], F32, tag=f"rf{tag}", name=f"rf{tag}")
        nc.vector.tensor_copy(out=r_f[:], in_=r_sb[:])
        invf = cp.tile([NCORES, 512], F32, tag=f"invf{tag}", name=f"invf{tag}")
        nc.vector.reciprocal_approx_fast(out=invf[:], in_=r_f[:])
        inv = cp.tile([NCORES, 512], BF16, tag=f"inv{tag}", name=f"inv{tag}")
        nc.vector.tensor_copy(out=inv[:], in_=invf[:])
        return inv

    def norm_block(cp, psC, cs, y_sb, yn_sb, inv, k, c0, c1):
        """yn_sb[:, k*512+c0 : k*512+c1] = y * broadcast(inv[:, c0:c1])."""
        pb = psC.tile([128, 512], F32, tag="psC", name="psBC")
        nc.tensor.matmul(
            pb[:, 0 : c1 - c0],
            cs["emat"][:, (k % 4) * 128 : (k % 4 + 1) * 128],
            inv[:, c0:c1],
            start=True,
            stop=True,
        )
        nc.vector.tensor_tensor(
            yn_sb[:, k * 512 + c0 : k * 512 + c1],
            y_sb[:, k * 512 + c0 : k * 512 + c1],
            pb[:, 0 : c1 - c0],
            mult,
        )

    def wo_rows(cp, psC, cs, yn_sb, acc, r, ks, first, last):
        """Wo contraction over k-chunks `ks` for output row-tile r."""
        for o in range(C // 512):
            pc = psC.tile([128, 512], F32, tag="psC", name="psC")
            for i, k in enumerate(ks):
                nc.tensor.matmul(
                    pc[:],
                    yn_sb[:, k * 512 + r * 128 : k * 512 + r * 128 + 128],
                    cs["wo"][:, k * C + o * 512 : k * C + (o + 1) * 512],
                    start=(i == 0),
                    stop=(last and i == len(ks) - 1),
                )
            if first:
                nc.tensor.matmul(
                    pc[:],
                    cs["ones"][0:1, :],
                    cs["bo"][0:1, o * 512 : (o + 1) * 512],
                    start=False,
                    stop=not last,
                )
            if not last:
                a = cp.tile([128, 512], F32, tag="acc", name="acc", bufs=4)
                nc.vector.tensor_copy(out=a[:], in_=pc[:])
                acc[(r, o)] = a
            else:
                osb = cp.tile([128, 512], F32, tag="osb", name="osb", bufs=3)
                if not first:
                    nc.vector.tensor_tensor(osb[:], pc[:], acc[(r, o)][:], add)
                else:
                    nc.vector.tensor_copy(out=osb[:], in_=pc[:])
                nc.sync.dma_start(
                    out=out[r * 128 : (r + 1) * 128, o * 512 : (o + 1) * 512],
                    in_=osb[:],
                )

    def stage_c_part_a(cp, psC, cs, acc, after):
        """After A2A_0: everything except h1-batch1 columns."""
        y_sb = cp.tile([128, 8 * 512], BF16, tag="ysb", name="ysb")
        r0_sb = cp.tile([NCORES, 512], BF16, tag="rsb0", name="rsb0")
        r1_sb = cp.tile([NCORES, 512], BF16, tag="rsb1", name="rsb1")
        dmas = []
        for s in range(NCORES):
            k, p = s // 2, s % 2
            # h0 k-chunks 0..3: full 512 columns
            dmas.append(nc.sync.dma_start(
                out=y_sb[p * D : p * D + D, k * 512 : (k + 1) * 512],
                in_=recv[0][s * SH : s * SH + D, 0:512],
            ))
            # h1 k-chunks 4..7: batch-0 half
            dmas.append(nc.sync.dma_start(
                out=y_sb[p * D : p * D + D, (k + 4) * 512 : (k + 4) * 512 + 256],
                in_=recv[0][s * SH : s * SH + D, 512:768],
            ))
            dmas.append(nc.sync.dma_start(
                out=r0_sb[s : s + 1, :],
                in_=recv[0][s * SH + D : s * SH + D + 1, 0:512],
            ))
            dmas.append(nc.sync.dma_start(
                out=r1_sb[s : s + 1, 0:256],
                in_=recv[0][s * SH + D : s * SH + D + 1, 512:768],
            ))
        unpack_dep(dmas, after)
        inv0 = make_inv(cp, r0_sb, "0")
        inv1 = make_inv(cp, r1_sb, "1")
        yn_sb = cp.tile([128, 8 * 512], BF16, tag="ynsb", name="ynsb")
        for k in range(4):
            norm_block(cp, psC, cs, y_sb, yn_sb, inv0, k, 0, 512)
        for k in range(4, 8):
            norm_block(cp, psC, cs, y_sb, yn_sb, inv1, k, 0, 256)
        # rows 0:256 (batch 0) have all 8 k-chunks now
        for r in (0, 1):
            wo_rows(cp, psC, cs, yn_sb, acc, r, list(range(8)), True, True)
        # rows 256:512 (batch 1): partial contraction over h0 chunks
        for r in (2, 3):
            wo_rows(cp, psC, cs, yn_sb, acc, r, [0, 1, 2, 3], True, False)
        return y_sb, yn_sb, r1_sb, inv1

    def stage_c_part_b(cp, psC, cs, acc, after, y_sb, yn_sb, r1_sb, inv1):
        """After A2A_1: h1-batch1 columns, then finish rows 256:512."""
        dmas = []
        for s in range(NCORES):
            k, p = s // 2, s % 2
            dmas.append(nc.sync.dma_start(
                out=y_sb[p * D : p * D + D,
                         (k + 4) * 512 + 256 : (k + 4) * 512 + 512],
                in_=recv[1][s * SH : s * SH + D, :],
            ))
            dmas.append(nc.sync.dma_start(
                out=r1_sb[s : s + 1, 256:512],
                in_=recv[1][s * SH + D : s * SH + D + 1, :],
            ))
        unpack_dep(dmas, after)
        inv1b = make_inv(cp, r1_sb, "1b")
        for k in range(4, 8):
            norm_block(cp, psC, cs, y_sb, yn_sb, inv1b, k, 256, 512)
        for r in (2, 3):
            wo_rows(cp, psC, cs, yn_sb, acc, r, [4, 5, 6, 7], False, True)

    with TileContext(nc) as tc:
        with tc.tile_pool(name="persist", bufs=1) as pp:
            cs = {}
            # weights first (needed with first x tiles)
            cs["wq"] = pp.tile([128, KT * COLS], BF16, tag="wq", name="wq")
            cs["wk"] = pp.tile([128, KT * COLS], BF16, tag="wk", name="wk")
            cs["wv"] = pp.tile([128, KT * COLS], BF16, tag="wv", name="wv")
            load_w3(cs["wq"], wq[:], COLS)
            load_w3(cs["wk"], wk[:], COLS)
            load_w3(cs["wv"], wv[:], COLS)

            with tc.tile_pool(name="xp", bufs=1) as xp:
                x_sb = []
                xq = [nc.sync, nc.scalar, nc.gpsimd]
                for k in range(KT):
                    xt = xp.tile([128, B * T], BF16, tag=f"x{k}", name=f"x{k}")
                    xq[k % 3].dma_start(out=xt[:], in_=xT[k * 128 : (k + 1) * 128, :])
                    x_sb.append(xt)

                # small constants (issued on scalar queue to not delay x)
                cs["ones"] = pp.tile([1, 128], BF16, tag="ones", name="ones")
                nc.vector.memset(cs["ones"][:], 1.0)
                cs["bqk"] = pp.tile([COLS, 2], F32, tag="bqk", name="bqk")
                nc.scalar.dma_start(out=cs["bqk"][:], in_=bqk[:])
                cs["bv"] = pp.tile([1, COLS], BF16, tag="bv", name="bv")
                nc.scalar.dma_start(out=cs["bv"][:], in_=bv[:])
                cs["bo"] = pp.tile([1, C], BF16, tag="bo", name="bo")
                nc.scalar.dma_start(out=cs["bo"][:], in_=bo[:])
                cs["mtri"] = pp.tile([128, 128], BF16, tag="mtri", name="mtri")
                nc.scalar.dma_start(out=cs["mtri"][:], in_=mtri[:])
                cs["emat"] = pp.tile([NCORES, C], BF16, tag="emat", name="emat")
                nc.scalar.dma_start(out=cs["emat"][:], in_=emat[:])
                cs["ident"] = pp.tile([128, 128], BF16, tag="ident", name="ident")
                make_identity(nc, cs["ident"][:])

                cs["qTd"] = [
                    pp.tile([128, B * T], BF16, tag=f"qTd{h}", name=f"qTd{h}")
                    for h in range(HL)
                ]
                cs["kTd"] = [
                    pp.tile([128, B * T], BF16, tag=f"kTd{h}", name=f"kTd{h}")
                    for h in range(HL)
                ]
                cs["v"] = pp.tile([128, B * NT * HL, VW], BF16, tag="v", name="v")
                nc.gpsimd.memset(cs["v"][:], 1.0)  # presets the ones columns

                # wo loaded last (not needed until stage C)
                cs["wo"] = pp.tile([128, KT * C], BF16, tag="wo", name="wo")
                nc.scalar.dma_start(
                    out=cs["wo"].rearrange("p (k w) -> p k w", k=KT),
                    in_=wo.rearrange("(k p) w -> p k w", k=KT),
                )

                with tc.tile_pool(name="psA", bufs=4, space="PSUM") as psA:
                    if "A" in STAGE:
                        stage_a(x_sb, psA, cs)

            acc = {}
            if "B" in STAGE:
                with tc.tile_pool(name="pt", bufs=40) as ptp, tc.tile_pool(
                    name="psS", bufs=1, space="PSUM"
                ) as psS, tc.tile_pool(
                    name="psY", bufs=1, space="PSUM"
                ) as psY, tc.tile_pool(
                    name="nrm", bufs=4
                ) as nrm, tc.tile_pool(
                    name="cp", bufs=1
                ) as cp, tc.tile_pool(
                    name="psC", bufs=2, space="PSUM"
                ) as psC:
                    send_instrs = []
                    if "2" in STAGE:
                        # warmup: absorb the ~11us first-collective trigger
                        # latency while projections run
                        nc.gpsimd.collective_compute(
                            "AllToAll",
                            mybir.AluOpType.bypass,
                            replica_groups=[list(range(NCORES))],
                            ins=[wsend[:]],
                            outs=[wrecv[:]],
                        )
                    stage_b_head(0, [0, 1], ptp, psS, psY, nrm, cs, send_instrs)
                    stage_b_head(1, [0], ptp, psS, psY, nrm, cs, send_instrs)
                    if "2" in STAGE:
                        a2a(0)
                    stage_b_head(1, [1], ptp, psS, psY, nrm, cs, send_instrs)
                    if "2" in STAGE:
                        a2a(1)
                        if "C" in STAGE:
                            st = stage_c_part_a(cp, psC, cs, acc, send_instrs[-1])
                            stage_c_part_b(cp, psC, cs, acc, send_instrs[-1], *st)
    nc.compile()
    return nc


def make_in_maps(x, mask, Wq, bq, Wk, bk, Wv, bv, Wo, bo):
    xT = np.ascontiguousarray(
        x.astype(np.float32).transpose(2, 0, 1).reshape(C, B * T)
    ).astype(NPBF16)
    mtri = np.where(
        np.arange(128)[:, None] > np.arange(128)[None, :], MASK_NEG, 0.0
    ).astype(NPBF16)
    # Wo rows permuted h-major: new row order = [core0 h0 d0..63, core1 h0, ...,
    # core7 h0, core0 h1, ..., core7 h1]
    perm = np.concatenate(
        [
            np.arange(c * COLS + h * D, c * COLS + h * D + D)
            for h in range(HL)
            for c in range(NCORES)
        ]
    )
    wo_b = np.ascontiguousarray(Wo[perm]).astype(NPBF16)
    # emat[s, c] = 1 iff shard s supplies the softmax normalizer for y-column
    # position c of the per-head k-chunk layout (chunk k = shards 2k, 2k+1)
    cols = np.arange(C)
    emat_np = (
        np.arange(NCORES)[:, None] == (2 * (cols // 128) + (cols % 128) // 64)
    ).astype(NPBF16)
    bo_b = bo.reshape(1, C).astype(NPBF16)
    in_maps = []
    for c in range(NCORES):
        cslice = slice(c * COLS, (c + 1) * COLS)
        in_maps.append(
            {
                "xT": xT,
                "wq": np.ascontiguousarray(Wq[:, cslice]).astype(NPBF16),
                "wk": np.ascontiguousarray(Wk[:, cslice]).astype(NPBF16),
                "wv": np.ascontiguousarray(Wv[:, cslice]).astype(NPBF16),
                "wo": wo_b,
                "bqk": np.stack([bq[cslice], bk[cslice]], axis=1).astype(
                    np.float32
                ),
                "bv": bv[cslice].reshape(1, COLS).astype(NPBF16),
                "bo": bo_b,
                "mtri": mtri,
                "emat": emat_np,
            }
        )
    return in_maps


_CACHED_NC = None


def run(inputs, trace=False, **kw):
    global _CACHED_NC
    if _CACHED_NC is None:
        _CACHED_NC = build_nc()
    in_maps = make_in_maps(**inputs)
    res = bass_utils.run_bass_kernel_spmd(
        _CACHED_NC, in_maps, core_ids=list(range(NCORES)), trace=trace, **kw
    )
    outs = [np.asarray(res.results[c]["out"]) for c in range(NCORES)]
    full = np.empty((B, T, C), np.float32)
    for j in range(NCORES):
        full[0, 256 * j : 256 * (j + 1)] = outs[j][0:256]
        full[1, 256 * j : 256 * (j + 1)] = outs[j][256:512]
    return full, res


def kernel(**inputs):
    full, _ = run(inputs, trace=False)
    return full


# revision 43
# speedup vs baseline: 1.1709x; 1.0875x over previous
"""Causal self-attention, tensor-parallel over heads across 8 TRN2 NeuronCores.

B=2, T=2048, C=1024, H=16 heads, D=64. Each core owns 2 heads (128 cols of C)
for both batches, computes QKV projections + causal attention + the softmax
normalizer (via a ones-column appended to v), then two per-head AllToAlls
convert the column-sharded attention output y^T into row shards so each core
computes a disjoint 512-row slice of the final Wo projection; the first A2A
and half the Wo contraction overlap the second head's attention compute.
bf16 matmuls, fp32 PSUM accumulation.
"""

import sys

sys.path.insert(0, "/opt/trn_rl_repo")

import numpy as np
import ml_dtypes

import concourse.bass as bass
import concourse.bacc as bacc
import concourse.mybir as mybir
from concourse.tile import TileContext
from concourse.masks import make_identity
from concourse.tile import add_dep_helper
from concourse import bass_utils

BF16 = mybir.dt.bfloat16
F32 = mybir.dt.float32
NPBF16 = ml_dtypes.bfloat16

B, T, C, H, D = 2, 2048, 1024, 16, 64
NCORES = 8
HL = H // NCORES          # heads per core = 2
COLS = HL * D             # 128 head-cols per core
KT = C // 128             # 8 contraction k-tiles
NCH = T // 512            # 4 query chunks of 512 per batch
NT = T // 128             # 16 key tiles of 128 per batch
VW = D + 1                # 65: v columns + ones column
SH = 72                   # A2A shard rows (64 y + 1 r + pad to whole 4KB)
ROWS_PER_CORE = B * T // NCORES  # 512 output rows per core

MASK_NEG = -60000.0
SCALE = 1.0 / np.sqrt(np.float32(D))
STAGE = "AB2C"
MASK_PE = True
MEMSET_GPS = True

Exp = mybir.ActivationFunctionType.Exp
Copy = mybir.ActivationFunctionType.Copy


def build_nc():
    nc = bacc.Bacc(
        "TRN2",
        target_bir_lowering=False,
        debug=False,
        enable_asserts=False,
        num_devices=NCORES,
    )
    xT = nc.dram_tensor("xT", [C, B * T], BF16, kind="ExternalInput")
    wq = nc.dram_tensor("wq", [C, COLS], BF16, kind="ExternalInput")
    wk = nc.dram_tensor("wk", [C, COLS], BF16, kind="ExternalInput")
    wv = nc.dram_tensor("wv", [C, COLS], BF16, kind="ExternalInput")
    # wo rows pre-permuted on host: h-major [h, core, 64]
    wo = nc.dram_tensor("wo", [C, C], BF16, kind="ExternalInput")
    bqk = nc.dram_tensor("bqk", [COLS, 2], F32, kind="ExternalInput")
    bv = nc.dram_tensor("bv", [1, COLS], BF16, kind="ExternalInput")
    bo = nc.dram_tensor("bo", [1, C], BF16, kind="ExternalInput")
    mtri = nc.dram_tensor("mtri", [128, 128], BF16, kind="ExternalInput")
    emat = nc.dram_tensor("emat", [NCORES, C], BF16, kind="ExternalInput")
    # 75/25 exchange: A2A_0 carries h0 (both batches) + h1 batch0; A2A_1 only
    # h1 batch1. Core j owns rows [256j:256j+256] of EACH batch, so every
    # chunk supplies all 8 destinations and shards stay uniform.
    send = [
        nc.dram_tensor("a2a_send0", [NCORES * SH, 768], BF16),
        nc.dram_tensor("a2a_send1", [NCORES * SH, 256], BF16),
    ]
    recv = [
        nc.dram_tensor("a2a_recv0", [NCORES * SH, 768], BF16),
        nc.dram_tensor("a2a_recv1", [NCORES * SH, 256], BF16),
    ]
    wsend = nc.dram_tensor("warm_send", [NCORES * 2, 512], BF16)
    wrecv = nc.dram_tensor("warm_recv", [NCORES * 2, 512], BF16)
    out = nc.dram_tensor("out", [ROWS_PER_CORE, C], F32, kind="ExternalOutput")

    add = mybir.AluOpType.add
    mult = mybir.AluOpType.mult

    def load_w3(dst, src, width):
        """One DMA: dst [128, KT*width] <- src [C, width] with k-tile blocks."""
        nc.sync.dma_start(
            out=dst.rearrange("p (k w) -> p k w", k=KT),
            in_=src.rearrange("(k p) w -> p k w", k=KT),
        )

    def stage_a(x_sb, psA, cs):
        for b in range(B):
            for proj, wsb, dsb, bcol in (
                ("q", cs["wq"], cs["qTd"], 0),
                ("k", cs["wk"], cs["kTd"], 1),
            ):
                for n in range(NCH):
                    ps = psA.tile([128, 512], F32, tag="psA", name="psA")
                    col = b * T + n * 512
                    for k in range(KT):
                        nc.tensor.matmul(
                            ps[:],
                            wsb[:, k * COLS : (k + 1) * COLS],
                            x_sb[k][:, col : col + 512],
                            start=(k == 0),
                            stop=(k == KT - 1),
                        )
                    # write each head into BOTH halves of its dup tile so
                    # score matmul pairs can use disjoint PE row-groups
                    for h in range(HL):
                        for half in range(2):
                            nc.vector.tensor_scalar(
                                dsb[h][half * D : half * D + D, col : col + 512],
                                ps[h * D : (h + 1) * D, :],
                                cs["bqk"][h * D : (h + 1) * D, bcol : bcol + 1],
                                None,
                                add,
                            )
            # v: natural layout [T-tile, cols]
            for m in range(NT):
                ps = psA.tile([128, COLS], F32, tag="psAv", name="psAv")
                col = b * T + m * 128
                for k in range(KT):
                    nc.tensor.matmul(
                        ps[:],
                        x_sb[k][:, col : col + 128],
                        cs["wv"][:, k * COLS : (k + 1) * COLS],
                        start=(k == 0),
                        stop=False,
                    )
                nc.tensor.matmul(
                    ps[:],
                    cs["ones"][0:1, :],
                    cs["bv"][0:1, :],
                    start=False,
                    stop=True,
                )
                # v_sb [128, B*NT*2, VW]; ones column (idx D) preset by memset
                vi = (b * NT + m) * HL
                nc.vector.tensor_copy(
                    out=cs["v"][:, vi : vi + HL, 0:D],
                    in_=ps[:, :].rearrange("p (h d) -> p h d", h=HL),
                )

    def stage_b_head(h, b_list, ptp, psS, psY, nrm, cs, send_instrs):
        for b in b_list:
            for n in range(NCH):
                qcol = b * T + n * 512
                qTd, kTd = cs["qTd"][h], cs["kTd"][h]
                pts = []
                for m0 in range(0, 4 * n, 2):
                    # two key tiles share one PSUM pair-tile and one exp; the
                    # two matmuls use disjoint PE row-groups and overlap
                    ps2 = psS.tile([128, 1024], F32, tag="psS2", name="psS2", bufs=2)
                    pt2 = ptp.tile([128, 1024], BF16, tag="pt2", name="pt2", bufs=20)
                    for i in range(2):
                        kcol = b * T + (m0 + i) * 128
                        nc.tensor.matmul(
                            ps2[:, i * 512 : (i + 1) * 512],
                            kTd[i * D : (i + 1) * D, kcol : kcol + 128],
                            qTd[i * D : (i + 1) * D, qcol : qcol + 512],
                            start=True,
                            stop=True,
                        )
                    nc.scalar.activation(pt2[:], ps2[:], Exp, scale=float(SCALE))
                    pts.append(pt2[:, 0:512])
                    pts.append(pt2[:, 512:1024])
                for m in range(4 * n, 4 * n + 4):
                    pt = ptp.tile([128, 512], BF16, tag="pt", name="pt")
                    ps = psS.tile([128, 512], F32, tag="psS", name="psS", bufs=1)
                    kcol = b * T + m * 128
                    if True:
                        j = m - 4 * n
                        # NOTE: all diag sub-MMs share one PSUM bank, so they
                        # must use ONE row-group (concurrent row-group writes
                        # to the same bank fault the PE)
                        for idx, sq in enumerate(range(j, 4)):
                            half = 0
                            nc.tensor.matmul(
                                ps[:, sq * 128 : (sq + 1) * 128],
                                kTd[half * D : (half + 1) * D, kcol : kcol + 128],
                                qTd[
                                    half * D : (half + 1) * D,
                                    qcol + sq * 128 : qcol + (sq + 1) * 128,
                                ],
                                start=True,
                                stop=(sq != j or not MASK_PE),
                            )
                            if sq == j and MASK_PE:
                                # causal mask of the diagonal block via PE:
                                # ps[:, j] += I.T @ mtri (adjacent in the
                                # accumulation group so no other start=True
                                # clears the region's has_written bits)
                                nc.tensor.matmul(
                                    ps[:, j * 128 : (j + 1) * 128],
                                    cs["ident"][:],
                                    cs["mtri"][:],
                                    start=False,
                                    stop=True,
                                )
                        if not MASK_PE:
                            nc.vector.tensor_tensor(
                                ps[:, j * 128 : (j + 1) * 128],
                                ps[:, j * 128 : (j + 1) * 128],
                                cs["mtri"][:],
                                add,
                            )
                        if j > 0:
                            (nc.gpsimd if MEMSET_GPS else nc.vector).memset(
                                pt[:, 0 : j * 128], 0.0
                            )
                        nc.scalar.activation(
                            pt[:, j * 128 :],
                            ps[:, j * 128 :],
                            Exp,
                            scale=float(SCALE),
                        )
                    pts.append(pt)

                # y^T (+ sum row) = v_aug^T @ P^T
                py = psY.tile([VW, 512], F32, tag="psY", name="psY")
                last = 4 * n + 3
                for m in range(4 * n + 4):
                    vi = (b * NT + m) * HL + h
                    nc.tensor.matmul(
                        py[:],
                        cs["v"][:, vi : vi + 1, :],
                        pts[m][:],
                        start=(m == 0),
                        stop=(m == last),
                    )
                # ship unnormalized y^T plus the r row; receive side divides.
                # chunk (b, n) rows [256p] belong to destination core 2n+p.
                yn = nrm.tile([VW, 512], BF16, tag="yn", name="yn")
                nc.vector.tensor_copy(out=yn[:], in_=py[:])
                for p in range(2):
                    dst = 2 * n + p
                    if h == 0:
                        o = send[0][dst * SH : dst * SH + VW, b * 256 : b * 256 + 256]
                    elif b == 0:
                        o = send[0][dst * SH : dst * SH + VW, 512:768]
                    else:
                        o = send[1][dst * SH : dst * SH + VW, 0:256]
                    si = nc.sync.dma_start(
                        out=o, in_=yn[:, p * 256 : (p + 1) * 256]
                    )
                    send_instrs.append(si)

    def a2a(h):
        nc.gpsimd.collective_compute(
            "AllToAll",
            mybir.AluOpType.bypass,
            replica_groups=[list(range(NCORES))],
            ins=[send[h][:]],
            outs=[recv[h][:]],
        )

    def unpack_dep(dmas, after):
        for dd in dmas:
            # keep unpack DMAs behind the attention sends on the Sync queue:
            # the scheduler otherwise queues them early and their
            # collective-completion wait blocks later sends
            add_dep_helper(
                dd.ins, after.ins, reason="unpack after attention sends"
            )

    def make_inv(cp, r_sb, tag):
        r_f = cp.tile([NCORES, 512], F32, tag=f"rf{tag}", name=f"rf{tag}")
        nc.vector.tensor_copy(out=r_f[:], in_=r_sb[:])
        invf = cp.tile([NCORES, 512], F32, tag=f"invf{tag}", name=f"invf{tag}")
        nc.vector.reciprocal_approx_fast(out=invf[:], in_=r_f[:])
        inv = cp.tile([NCORES, 512], BF16, tag=f"inv{tag}", name=f"inv{tag}")
        nc.vector.tensor_copy(out=inv[:], in_=invf[:])
        return inv

    def norm_block(cp, psC, cs, y_sb, yn_sb, inv, k, c0, c1):
        """yn_sb[:, k*512+c0 : k*512+c1] = y * broadcast(inv[:, c0:c1])."""
        pb = psC.tile([128, 512], F32, tag="psB", name="psBC", bufs=2)
        nc.tensor.matmul(
            pb[:, 0 : c1 - c0],
            cs["emat"][:, (k % 4) * 128 : (k % 4 + 1) * 128],
            inv[:, c0:c1],
            start=True,
            stop=True,
        )
        nc.vector.tensor_tensor(
            yn_sb[:, k * 512 + c0 : k * 512 + c1],
            y_sb[:, k * 512 + c0 : k * 512 + c1],
            pb[:, 0 : c1 - c0],
            mult,
        )

    def wo_rows(cp, psC, cs, yn_sb, acc, r, ks, first, last):
        """Wo contraction over k-chunks `ks` for output row-tile r."""
        for o in range(C // 512):
            pc = psC.tile([128, 512], F32, tag="psC", name="psC", bufs=4)
            for i, k in enumerate(ks):
                nc.tensor.matmul(
                    pc[:],
                    yn_sb[:, k * 512 + r * 128 : k * 512 + r * 128 + 128],
                    cs["wo"][:, k * C + o * 512 : k * C + (o + 1) * 512],
                    start=(i == 0),
                    stop=(last and i == len(ks) - 1),
                )
            if first:
                nc.tensor.matmul(
                    pc[:],
                    cs["ones"][0:1, :],
                    cs["bo"][0:1, o * 512 : (o + 1) * 512],
                    start=False,
                    stop=not last,
                )
            if not last:
                a = cp.tile([128, 512], F32, tag="acc", name="acc", bufs=4)
                nc.vector.tensor_copy(out=a[:], in_=pc[:])
                acc[(r, o)] = a
            else:
                osb = cp.tile([128, 512], F32, tag="osb", name="osb", bufs=3)
                if not first:
                    nc.vector.tensor_tensor(osb[:], pc[:], acc[(r, o)][:], add)
                else:
                    nc.vector.tensor_copy(out=osb[:], in_=pc[:])
                nc.sync.dma_start(
                    out=out[r * 128 : (r + 1) * 128, o * 512 : (o + 1) * 512],
                    in_=osb[:],
                )

    def stage_c_part_a(cp, psC, cs, acc, after):
        """After A2A_0: everything except h1-batch1 columns."""
        y_sb = cp.tile([128, 8 * 512], BF16, tag="ysb", name="ysb")
        r0_sb = cp.tile([NCORES, 512], BF16, tag="rsb0", name="rsb0")
        r1_sb = cp.tile([NCORES, 512], BF16, tag="rsb1", name="rsb1")
        # recv0 viewed as [k, parity, 72, c]
        rv = recv[0].rearrange("(k p2 dr) c -> p2 dr k c", k=4, p2=2)
        dmas = []
        for p in range(2):
            # h0 k-chunks 0..3: full 512 columns, one strided DMA per parity
            dmas.append(nc.sync.dma_start(
                out=y_sb[p * D : p * D + D, 0 : 4 * 512].rearrange(
                    "d (k c) -> d k c", k=4
                ),
                in_=rv[p, 0:D, :, 0:512],
            ))
            # h1 k-chunks 4..7: batch-0 half
            dmas.append(nc.sync.dma_start(
                out=y_sb[p * D : p * D + D, 4 * 512 : 8 * 512].rearrange(
                    "d (k c) -> d k c", k=4
                )[:, :, 0:256],
                in_=rv[p, 0:D, :, 512:768],
            ))
        rr = recv[0].rearrange("(s dr) c -> s dr c", s=NCORES)
        dmas.append(nc.sync.dma_start(
            out=r0_sb[:].rearrange("s (o c) -> s o c", o=1),
            in_=rr[:, D : D + 1, 0:512],
        ))
        dmas.append(nc.sync.dma_start(
            out=r1_sb[:, 0:256].rearrange("s (o c) -> s o c", o=1),
            in_=rr[:, D : D + 1, 512:768],
        ))
        unpack_dep(dmas, after)
        inv0 = make_inv(cp, r0_sb, "0")
        inv1 = make_inv(cp, r1_sb, "1")
        yn_sb = cp.tile([128, 8 * 512], BF16, tag="ynsb", name="ynsb")
        for k in range(4):
            norm_block(cp, psC, cs, y_sb, yn_sb, inv0, k, 0, 512)
        for k in range(4, 8):
            norm_block(cp, psC, cs, y_sb, yn_sb, inv1, k, 0, 256)
        # rows 0:256 (batch 0) have all 8 k-chunks now
        for r in (0, 1):
            wo_rows(cp, psC, cs, yn_sb, acc, r, list(range(8)), True, True)
        # rows 256:512 (batch 1): partial contraction over h0 chunks
        for r in (2, 3):
            wo_rows(cp, psC, cs, yn_sb, acc, r, [0, 1, 2, 3], True, False)
        return y_sb, yn_sb, r1_sb, inv1

    def stage_c_part_b(cp, psC, cs, acc, after, y_sb, yn_sb, r1_sb, inv1):
        """After A2A_1: h1-batch1 columns, then finish rows 256:512."""
        rv1 = recv[1].rearrange("(k p2 dr) c -> p2 dr k c", k=4, p2=2)
        rr1 = recv[1].rearrange("(s dr) c -> s dr c", s=NCORES)
        dmas = []
        for p in range(2):
            dmas.append(nc.sync.dma_start(
                out=y_sb[p * D : p * D + D, 4 * 512 : 8 * 512].rearrange(
                    "d (k c) -> d k c", k=4
                )[:, :, 256:512],
                in_=rv1[p, 0:D, :, :],
            ))
        dmas.append(nc.sync.dma_start(
            out=r1_sb[:, 256:512].rearrange("s (o c) -> s o c", o=1),
            in_=rr1[:, D : D + 1, :],
        ))
        unpack_dep(dmas, after)
        inv1b = make_inv(cp, r1_sb, "1b")
        for k in range(4, 8):
            norm_block(cp, psC, cs, y_sb, yn_sb, inv1b, k, 256, 512)
        for r in (2, 3):
            wo_rows(cp, psC, cs, yn_sb, acc, r, [4, 5, 6, 7], False, True)

    with TileContext(nc) as tc:
        with tc.tile_pool(name="persist", bufs=1) as pp:
            cs = {}
            # weights first (needed with first x tiles)
            cs["wq"] = pp.tile([128, KT * COLS], BF16, tag="wq", name="wq")
            cs["wk"] = pp.tile([128, KT * COLS], BF16, tag="wk", name="wk")
            cs["wv"] = pp.tile([128, KT * COLS], BF16, tag="wv", name="wv")
            load_w3(cs["wq"], wq[:], COLS)
            load_w3(cs["wk"], wk[:], COLS)
            load_w3(cs["wv"], wv[:], COLS)

            with tc.tile_pool(name="xp", bufs=1) as xp:
                x_sb = []
                xq = [nc.sync, nc.scalar, nc.gpsimd]
                for k in range(KT):
                    xt = xp.tile([128, B * T], BF16, tag=f"x{k}", name=f"x{k}")
                    xq[k % 3].dma_start(out=xt[:], in_=xT[k * 128 : (k + 1) * 128, :])
                    x_sb.append(xt)

                # small constants (issued on scalar queue to not delay x)
                cs["ones"] = pp.tile([1, 128], BF16, tag="ones", name="ones")
                nc.vector.memset(cs["ones"][:], 1.0)
                cs["bqk"] = pp.tile([COLS, 2], F32, tag="bqk", name="bqk")
                nc.scalar.dma_start(out=cs["bqk"][:], in_=bqk[:])
                cs["bv"] = pp.tile([1, COLS], BF16, tag="bv", name="bv")
                nc.scalar.dma_start(out=cs["bv"][:], in_=bv[:])
                cs["bo"] = pp.tile([1, C], BF16, tag="bo", name="bo")
                nc.scalar.dma_start(out=cs["bo"][:], in_=bo[:])
                cs["mtri"] = pp.tile([128, 128], BF16, tag="mtri", name="mtri")
                nc.scalar.dma_start(out=cs["mtri"][:], in_=mtri[:])
                cs["emat"] = pp.tile([NCORES, C], BF16, tag="emat", name="emat")
                nc.scalar.dma_start(out=cs["emat"][:], in_=emat[:])
                cs["ident"] = pp.tile([128, 128], BF16, tag="ident", name="ident")
                make_identity(nc, cs["ident"][:])

                cs["qTd"] = [
                    pp.tile([128, B * T], BF16, tag=f"qTd{h}", name=f"qTd{h}")
                    for h in range(HL)
                ]
                cs["kTd"] = [
                    pp.tile([128, B * T], BF16, tag=f"kTd{h}", name=f"kTd{h}")
                    for h in range(HL)
                ]
                cs["v"] = pp.tile([128, B * NT * HL, VW], BF16, tag="v", name="v")
                nc.gpsimd.memset(cs["v"][:], 1.0)  # presets the ones columns

                # wo loaded last (not needed until stage C)
                cs["wo"] = pp.tile([128, KT * C], BF16, tag="wo", name="wo")
                nc.scalar.dma_start(
                    out=cs["wo"].rearrange("p (k w) -> p k w", k=KT),
                    in_=wo.rearrange("(k p) w -> p k w", k=KT),
                )

                with tc.tile_pool(name="psA", bufs=4, space="PSUM") as psA:
                    if "A" in STAGE:
                        stage_a(x_sb, psA, cs)

            acc = {}
            if "B" in STAGE:
                with tc.tile_pool(name="pt", bufs=40) as ptp, tc.tile_pool(
                    name="psS", bufs=1, space="PSUM"
                ) as psS, tc.tile_pool(
                    name="psY", bufs=1, space="PSUM"
                ) as psY, tc.tile_pool(
                    name="nrm", bufs=4
                ) as nrm:
                    send_instrs = []
                    if "2" in STAGE:
                        # warmup: absorb the ~11us first-collective trigger
                        # latency while projections run
                        nc.gpsimd.collective_compute(
                            "AllToAll",
                            mybir.AluOpType.bypass,
                            replica_groups=[list(range(NCORES))],
                            ins=[wsend[:]],
                            outs=[wrecv[:]],
                        )
                    stage_b_head(0, [0, 1], ptp, psS, psY, nrm, cs, send_instrs)
                    stage_b_head(1, [0], ptp, psS, psY, nrm, cs, send_instrs)
                    if "2" in STAGE:
                        a2a(0)
                    stage_b_head(1, [1], ptp, psS, psY, nrm, cs, send_instrs)
                    if "2" in STAGE:
                        a2a(1)
                if "2" in STAGE and "C" in STAGE:
                    with tc.tile_pool(name="cp", bufs=1) as cp, tc.tile_pool(
                        name="psC", bufs=2, space="PSUM"
                    ) as psC:
                        st = stage_c_part_a(cp, psC, cs, acc, send_instrs[-1])
                        stage_c_part_b(cp, psC, cs, acc, send_instrs[-1], *st)
    nc.compile()
    return nc


def make_in_maps(x, mask, Wq, bq, Wk, bk, Wv, bv, Wo, bo):
    xT = np.ascontiguousarray(
        x.astype(np.float32).transpose(2, 0, 1).reshape(C, B * T)
    ).astype(NPBF16)
    mtri = np.where(
        np.arange(128)[:, None] > np.arange(128)[None, :], MASK_NEG, 0.0
    ).astype(NPBF16)
    # Wo rows permuted h-major: new row order = [core0 h0 d0..63, core1 h0, ...,
    # core7 h0, core0 h1, ..., core7 h1]
    perm = np.concatenate(
        [
            np.arange(c * COLS + h * D, c * COLS + h * D + D)
            for h in range(HL)
            for c in range(NCORES)
        ]
    )
    wo_b = np.ascontiguousarray(Wo[perm]).astype(NPBF16)
    # emat[s, c] = 1 iff shard s supplies the softmax normalizer for y-column
    # position c of the per-head k-chunk layout (chunk k = shards 2k, 2k+1)
    cols = np.arange(C)
    emat_np = (
        np.arange(NCORES)[:, None] == (2 * (cols // 128) + (cols % 128) // 64)
    ).astype(NPBF16)
    bo_b = bo.reshape(1, C).astype(NPBF16)
    in_maps = []
    for c in range(NCORES):
        cslice = slice(c * COLS, (c + 1) * COLS)
        in_maps.append(
            {
                "xT": xT,
                "wq": np.ascontiguousarray(Wq[:, cslice]).astype(NPBF16),
                "wk": np.ascontiguousarray(Wk[:, cslice]).astype(NPBF16),
                "wv": np.ascontiguousarray(Wv[:, cslice]).astype(NPBF16),
                "wo": wo_b,
                "bqk": np.stack([bq[cslice], bk[cslice]], axis=1).astype(
                    np.float32
                ),
                "bv": bv[cslice].reshape(1, COLS).astype(NPBF16),
                "bo": bo_b,
                "mtri": mtri,
                "emat": emat_np,
            }
        )
    return in_maps


_CACHED_NC = None


def run(inputs, trace=False, **kw):
    global _CACHED_NC
    if _CACHED_NC is None:
        _CACHED_NC = build_nc()
    in_maps = make_in_maps(**inputs)
    res = bass_utils.run_bass_kernel_spmd(
        _CACHED_NC, in_maps, core_ids=list(range(NCORES)), trace=trace, **kw
    )
    outs = [np.asarray(res.results[c]["out"]) for c in range(NCORES)]
    full = np.empty((B, T, C), np.float32)
    for j in range(NCORES):
        full[0, 256 * j : 256 * (j + 1)] = outs[j][0:256]
        full[1, 256 * j : 256 * (j + 1)] = outs[j][256:512]
    return full, res


def kernel(**inputs):
    full, _ = run(inputs, trace=False)
    return full
